# revision 1
# baseline (speedup 1.0000x reference)
"""Behavior-specific PFF (MoE-routed FFN + residual + LayerNorm) on 8 TRN2 cores.

Strategy: expert-parallel with host-side token dispatch.
  - b_seq in [0..4]; 0 = padding (output zeros). Each of the 4 behaviors gets
    2 NeuronCores; its tokens are split between them.
  - Host gathers each core's tokens transposed ([D, C], matmul rhs layout),
    padded to a common capacity C (multiple of 256).
  - Device (per core, same SPMD program, different data):
      hT[f, t] = relu(W1.T chunks @ xT chunks + b1)    (PE fp32r + ACT relu)
      y[t, d]  = hT slices.T @ W2.T chunks             (PE fp32r, psum accum)
      r        = y + (x + b2)                          (DVE, x via [C,D] copy)
      mean,var = bn_stats/bn_aggr(r)                   (DVE)
      out      = (r - mean)/sqrt(var+eps) [*gamma+beta] (DVE tensor_scalar)
    mm1 runs one 512-token block ahead of mm2 (depth-2 software pipeline) so
    the serial DMA queue can deliver W2 before mm2 needs it.
  - Host scatters per-core outputs back to the full (B, T, D) tensor.

fp32r = Trainium's full-rate fp32 matmul mode (~1e-4 rel err vs fp32).
"""
import math
import time
import numpy as np
from contextlib import ExitStack

import concourse.bacc as bacc
import concourse.tile as tile
import concourse.mybir as mybir
from concourse import bass_utils

F32 = mybir.dt.float32
F32R = mybir.dt.float32r
AF = mybir.ActivationFunctionType
ALU = mybir.AluOpType

D = 512
F = 2048
NB = 4
EPS = 1e-5
NCORES = 8

# test.py introspection hooks (harness never touches these)
LAST_RUN = {}


def _build_nc(C: int, apply_gb: bool, apply_b1: bool, apply_b2: bool,
              mm_dt=F32R, repeats=1):
    """Build + compile the single-core program (run SPMD on 8 cores)."""
    nc = bacc.Bacc("TRN2", target_bir_lowering=False, debug=False,
                   num_devices=NCORES)
    xt_d = nc.dram_tensor("xt", [D, C], mm_dt, kind="ExternalInput").ap()
    xr_d = nc.dram_tensor("xr", [C, D], F32, kind="ExternalInput").ap()
    w1t_d = nc.dram_tensor("w1t", [D, F], mm_dt, kind="ExternalInput").ap()
    w2t_d = nc.dram_tensor("w2t", [F, D], mm_dt, kind="ExternalInput").ap()
    if apply_b1:
        b1c_d = nc.dram_tensor("b1c", [128, F // 128], F32,
                               kind="ExternalInput").ap()
    if apply_gb:
        gb_d = nc.dram_tensor("gb", [128, D], F32, kind="ExternalInput").ap()
        bb_d = nc.dram_tensor("bb", [128, D], F32, kind="ExternalInput").ap()
    y_d = nc.dram_tensor("y", [C, D], F32, kind="ExternalOutput").ap()

    KC1 = D // 128    # k-chunks for x @ W1.T (4)
    KC2 = F // 128    # k-chunks for h @ W2.T (16)
    NP = KC2 // 2     # f-tile pairs (8)

    # Token blocks of 256..512 (multiples of 128). N >= 256 keeps fp32r
    # matmuls at full rate, so a 128 remainder is folded as 384+256.
    sizes = []
    rem = C
    while rem > 640:
        sizes.append(512)
        rem -= 512
    if rem == 640:
        sizes += [384, 256]
    elif rem == 128:
        if sizes:
            sizes[-1] = 384
            sizes.append(256)
        else:
            sizes.append(128)
    else:
        sizes.append(rem)
    blocks = []
    t0 = 0
    for nb in sizes:
        blocks.append((t0, nb))
        t0 += nb

    with tile.TileContext(nc) as tc, ExitStack() as ctx:
        wp = ctx.enter_context(tc.tile_pool(name="wp", bufs=1))
        xtp = ctx.enter_context(tc.tile_pool(name="xtp", bufs=2))
        htp = ctx.enter_context(tc.tile_pool(name="htp", bufs=2))
        xrp = ctx.enter_context(tc.tile_pool(name="xrp", bufs=3))
        rp = ctx.enter_context(tc.tile_pool(name="rp", bufs=4))
        outp = ctx.enter_context(tc.tile_pool(name="outp", bufs=4))
        sp = ctx.enter_context(tc.tile_pool(name="sp", bufs=6))
        ps1 = ctx.enter_context(tc.tile_pool(name="ps1", bufs=3, space="PSUM"))
        ps2 = ctx.enter_context(tc.tile_pool(name="ps2", bufs=2, space="PSUM"))

        # One batched DMA per transfer: the HWDGE queue costs a fixed slot
        # per DMA instruction, so fewer/bigger transfers supply faster.
        xt_r = xt_d.rearrange("(k p) c -> p k c", p=128)   # [128, KC1, C]
        xr_r = xr_d.rearrange("(t p) d -> p t d", p=128)   # [128, C/128, D]
        w1_r = w1t_d.rearrange("(k p) f -> p k f", p=128)  # [128, KC1, F]
        w2_r = w2t_d.rearrange("(k p) d -> p k d", p=128)  # [128, KC2, D]

        def load_xt(t0, nb):
            xt = xtp.tile([128, KC1, nb], mm_dt, name="xt", tag="xt")
            nc.sync.dma_start(xt[:], xt_r[:, :, t0:t0 + nb])
            return xt

        def load_xr(t0, nb):
            xr = xrp.tile([128, nb // 128, D], F32, name="xr", tag="xr")
            nc.sync.dma_start(xr[:], xr_r[:, t0 // 128:(t0 + nb) // 128, :])
            return xr

        # DMA issue order matters: the HWDGE queue drains serially in issue
        # order, so DMAs are emitted in deadline order. mm1 runs one block
        # ahead of mm2 (software pipeline depth 2), which moves the W2
        # deadline late enough for the full prologue to fit the DMA supply.
        nblk = len(blocks)
        xt_tiles, xr_tiles = {}, {}
        ht_tiles = {}
        w1_sb = wp.tile([128, KC1, F], mm_dt, name="w1_sb")
        w2_sb = wp.tile([128, KC2, D], mm_dt, name="w2_sb")

        # PE warm-up: fp32 dummy matmuls on a Pool-memset tile (no DMA, so
        # the critical xt0/W1 transfers aren't delayed) keep the PE busy
        # through the DMA prologue so the clock ramp (HAM) finishes before
        # the first real matmul. Plain fp32 avoids the f32r-producer check.
        warm = wp.tile([128, 128], F32, name="warm")
        nc.gpsimd.memset(warm[:], 0.001)
        xt_tiles[0] = load_xt(*blocks[0])
        # W1 in four deadline-matched slices: a small leading one unblocks
        # the first f-pair; the rest stream in under mm1 of block 0.
        nc.sync.dma_start(w1_sb[:, :, 0:256], w1_r[:, :, 0:256])
        pwarm = ps2.tile([128, 128], F32, name="pwarm", tag="p2")
        NWARM = 42
        for wi in range(NWARM):
            nc.tensor.matmul(pwarm[:, 0:64], warm[:], warm[:, 0:64],
                             start=(wi == 0), stop=(wi == NWARM - 1))
        eps_sb = wp.tile([128, 1], F32, name="eps_sb")
        nc.vector.memset(eps_sb[:], EPS)
        if apply_b1:
            b1_sb = wp.tile([128, KC2], F32, name="b1_sb")
            nc.sync.dma_start(b1_sb[:], b1c_d[:])
        if apply_gb:
            gb_sb = wp.tile([128, D], F32, name="gb_sb")
            nc.sync.dma_start(gb_sb[:], gb_d[:])
            bb_sb = wp.tile([128, D], F32, name="bb_sb")
            nc.sync.dma_start(bb_sb[:], bb_d[:])
        nc.sync.dma_start(w1_sb[:, :, 256:1024], w1_r[:, :, 256:1024])
        nc.sync.dma_start(w1_sb[:, :, 1024:1664], w1_r[:, :, 1024:1664])
        nc.sync.dma_start(w1_sb[:, :, 1664:2048], w1_r[:, :, 1664:2048])
        if nblk > 1:
            xt_tiles[1] = load_xt(*blocks[1])
        nc.sync.dma_start(w2_sb[:, 0:8, :], w2_r[:, 0:8, :])
        nc.sync.dma_start(w2_sb[:, 8:16, :], w2_r[:, 8:16, :])
        xr_tiles[0] = load_xr(*blocks[0])

        def emit_mm1(bi):
            t0, nb = blocks[bi]
            xt_t = xt_tiles[bi]
            ht_t = []
            for j in range(NP):
                # [128, 2, 512] regardless of nb: each si-slice must start on
                # a PSUM bank boundary (matmul output can't straddle banks).
                p1 = ps1.tile([128, 2, 512], F32, name=f"p1_{j}", tag="p1")
                for si in range(2):
                    f = 2 * j + si
                    for k in range(KC1):
                        nc.tensor.matmul(p1[:, si, 0:nb],
                                         w1_sb[:, k, 128 * f:128 * (f + 1)],
                                         xt_t[:, k, :],
                                         start=(k == 0), stop=(k == KC1 - 1))
                ht = htp.tile([128, 2, nb], mm_dt, name=f"ht_{j}", tag=f"ht{j}")
                if apply_b1:
                    for si in range(2):
                        f = 2 * j + si
                        nc.scalar.activation(ht[:, si, :], p1[:, si, 0:nb],
                                             AF.Relu, bias=b1_sb[:, f:f + 1])
                else:
                    nc.scalar.activation(ht[:, :, :], p1[:, :, 0:nb], AF.Relu)
                ht_t.append(ht)
            ht_tiles[bi] = ht_t

        def emit_mm2(bi):
            t0, nb = blocks[bi]
            ht_t = ht_tiles.pop(bi)
            xr_t_blk = xr_tiles.pop(bi)
            for tt in range(nb // 128):
                sl = slice(128 * tt, 128 * (tt + 1))
                p2 = ps2.tile([128, D], F32, name="p2", tag="p2")
                for k in range(KC2):
                    nc.tensor.matmul(p2[:], ht_t[k // 2][:, k % 2, sl],
                                     w2_sb[:, k, :],
                                     start=(k == 0), stop=(k == KC2 - 1))
                r = rp.tile([128, D], F32, name="r", tag="r")
                nc.vector.tensor_add(r[:], p2[:], xr_t_blk[:, tt, :])
                st6 = sp.tile([128, 6], F32, name="st6", tag="st6")
                nc.vector.bn_stats(st6[:], r[:])
                mv = sp.tile([128, 2], F32, name="mv", tag="mv")
                nc.vector.bn_aggr(mv[:], st6[:])
                stdt = sp.tile([128, 1], F32, name="stdt", tag="stdt")
                nc.scalar.activation(stdt[:], mv[:, 1:2], AF.Sqrt,
                                     bias=eps_sb[:])
                rstd = sp.tile([128, 1], F32, name="rstd", tag="rstd")
                nc.vector.reciprocal(rstd[:], stdt[:])
                nbias = sp.tile([128, 1], F32, name="nbias", tag="nbias")
                nc.vector.scalar_tensor_tensor(nbias[:], mv[:, 0:1], -1.0,
                                               rstd[:], op0=ALU.mult,
                                               op1=ALU.mult)
                o = outp.tile([128, D], F32, name="o", tag="o")
                if apply_gb:
                    t1 = rp.tile([128, D], F32, name="t1", tag="t1")
                    nc.vector.tensor_scalar(t1[:], r[:], rstd[:], nbias[:],
                                            op0=ALU.mult, op1=ALU.add)
                    t2 = rp.tile([128, D], F32, name="t2", tag="t2")
                    nc.vector.tensor_mul(t2[:], t1[:], gb_sb[:])
                    nc.vector.tensor_add(o[:], t2[:], bb_sb[:])
                else:
                    nc.vector.tensor_scalar(o[:], r[:], rstd[:], nbias[:],
                                            op0=ALU.mult, op1=ALU.add)
                nc.sync.dma_start(y_d[t0 + 128 * tt:t0 + 128 * (tt + 1), :],
                                  o[:])

        for rep in range(repeats):
            for bi in range(nblk):
                emit_mm1(bi)
                if bi + 1 < nblk:
                    xr_tiles[bi + 1] = load_xr(*blocks[bi + 1])
                if bi >= 1:
                    emit_mm2(bi - 1)
                if bi + 2 < nblk:
                    xt_tiles[bi + 2] = load_xt(*blocks[bi + 2])
            emit_mm2(nblk - 1)
            if rep + 1 < repeats:
                xt_tiles[0] = load_xt(*blocks[0])
                if nblk > 1:
                    xt_tiles[1] = load_xt(*blocks[1])
                xr_tiles[0] = load_xr(*blocks[0])

    nc.compile()
    return nc


def kernel(x, b_seq, W1, b1, W2, b2, gamma, beta):
    x = np.asarray(x, dtype=np.float32)
    b_seq_np = np.asarray(b_seq)
    W1 = np.asarray(W1, dtype=np.float32)
    b1 = np.asarray(b1, dtype=np.float32)
    W2 = np.asarray(W2, dtype=np.float32)
    b2 = np.asarray(b2, dtype=np.float32)
    gamma = np.asarray(gamma, dtype=np.float32)
    beta = np.asarray(beta, dtype=np.float32)

    B, T, D_ = x.shape
    assert D_ == D and W1.shape == (NB, F, D)
    tokens = np.ascontiguousarray(x.reshape(-1, D))
    bs = b_seq_np.reshape(-1).astype(np.int64)

    # Token dispatch: expert e -> cores 2e and 2e+1.
    idx_per_core = []
    for e in range(NB):
        idx = np.nonzero(bs == e + 1)[0]
        h = (len(idx) + 1) // 2
        idx_per_core.append(idx[:h])
        idx_per_core.append(idx[h:])
    cmax = max(len(i) for i in idx_per_core)
    out = np.zeros_like(tokens)
    if cmax == 0:
        return out.reshape(B, T, D).astype(x.dtype)
    C = max(256, int(math.ceil(cmax / 128.0)) * 128)

    apply_gb = not (np.all(gamma == 1.0) and np.all(beta == 0.0))
    apply_b1 = bool(np.any(b1 != 0.0))
    apply_b2 = bool(np.any(b2 != 0.0))
    nc = _build_nc(C, apply_gb, apply_b1, apply_b2)

    in_maps = []
    for core in range(NCORES):
        e = core // 2
        idx = idx_per_core[core]
        n = len(idx)
        xt = np.zeros((D, C), np.float32)
        xt[:, :n] = tokens[idx].T
        xr = np.zeros((C, D), np.float32)
        xr[:n] = tokens[idx] + b2[e]
        m = {
            "xt": xt,
            "xr": xr,
            "w1t": np.ascontiguousarray(W1[e].T),
            "w2t": np.ascontiguousarray(W2[e].T),
        }
        if apply_b1:
            m["b1c"] = np.ascontiguousarray(b1[e].reshape(F // 128, 128).T)
        if apply_gb:
            m["gb"] = np.ascontiguousarray(
                np.broadcast_to(gamma[e], (128, D)).astype(np.float32))
            m["bb"] = np.ascontiguousarray(
                np.broadcast_to(beta[e], (128, D)).astype(np.float32))
        in_maps.append(m)

    # Transient NRT_EXEC_UNIT_UNRECOVERABLE states heal after a cooldown;
    # retry rather than failing the whole call.
    last_exc = None
    for attempt in range(4):
        try:
            res = bass_utils.run_bass_kernel_spmd(
                nc, in_maps, core_ids=list(range(NCORES)))
            break
        except Exception as e:
            last_exc = e
            if attempt == 3:
                raise
            time.sleep(75)
    else:
        raise last_exc

    for core in range(NCORES):
        idx = idx_per_core[core]
        if len(idx):
            out[idx] = res.results[core]["y"][:len(idx)]

    LAST_RUN["nc"] = nc
    LAST_RUN["in_maps"] = in_maps
    return out.reshape(B, T, D).astype(x.dtype)

